# revision 27
# baseline (speedup 1.0000x reference)
"""Multi-head causal attention (B=4, T=2048, C=1024, H=16) on 8 TRN2 NeuronCores.

Sharding: core c <-> (batch b = c//2, head-group hg = c%2 of 8 heads).
Each core: QKV projection for its batch tokens / its 8 heads, flash-style
causal attention in transposed-score layout with unsafe softmax (row sums via
a ones-column in V), and a partial out-projection over its 512 head-dims.
Host pre-transposes/casts inputs (bf16) and sums the two partials per batch.

Matmuls in bf16 (full PE rate, FWL weight loads); accumulation and softmax in
fp32 (PSUM); normalization uses batched reciprocal so PSUM banks free fast.
"""

import numpy as np
import ml_dtypes

import concourse.bass as bass
import concourse.mybir as mybir
import concourse.tile as tile
from concourse import bacc, bass_utils

B, T, C = 4, 2048, 1024
N_HEAD = 16
HEAD_DIM = 64
SCALE = 1.0 / np.sqrt(HEAD_DIM)

HG = 8                  # heads per core
NPAIR = HG // 2         # head pairs per core (a pair shares a 128-partition block)
NTT = T // 128          # token tiles (16)
NTS = T // 512          # token supertiles (4)
NCC = C // 128          # contraction chunks (8)

F32 = mybir.dt.float32
BF16 = mybir.dt.bfloat16

_CACHE = {}


def qkv_projection(nc, tc, xT_t, wqk_t, wv_t, q_sb, k_sb, v_sb):
    with tc.tile_pool(name="wqk_p", bufs=1) as wqk_p, \
         tc.tile_pool(name="wv_p", bufs=1) as wv_p, \
         tc.tile_pool(name="x_p", bufs=3) as x_p, \
         tc.tile_pool(name="ps_qk", bufs=2, space="PSUM") as ps_qk, \
         tc.tile_pool(name="ps_v", bufs=2, space="PSUM") as ps_v:
        w_qk = wqk_p.tile([128, NCC, 1024], BF16)
        nc.sync.dma_start(out=w_qk[:, 0, :], in_=wqk_t[:, 0, :])
        x0 = x_p.tile([128, NCC, 512], BF16, tag="x0")
        nc.sync.dma_start(out=x0, in_=xT_t[:, :, 0:512])
        for cc in range(1, NCC):
            nc.sync.dma_start(out=w_qk[:, cc, :], in_=wqk_t[:, cc, :])
        w_v = wv_p.tile([128, NCC, 512], BF16)
        nc.sync.dma_start(out=w_v, in_=wv_t)
        x_ts = [x0]

        for ts in range(NTS):
            if ts > 0:
                x_t = x_p.tile([128, NCC, 512], BF16, tag=f"x{ts % 3}")
                nc.sync.dma_start(out=x_t,
                                  in_=xT_t[:, :, ts * 512:(ts + 1) * 512])
            else:
                x_t = x_ts[0]

            # q and k, feature-major: out [f128, tok512]
            for f in range(2 * NPAIR):  # 0..3 q-pairs, 4..7 k-pairs
                pq = ps_qk.tile([128, 512], F32)
                for cc in range(NCC):
                    nc.tensor.matmul(
                        pq,
                        w_qk[:, cc, f * 128:(f + 1) * 128],
                        x_t[:, cc, :],
                        start=(cc == 0), stop=(cc == NCC - 1),
                    )
                dst = q_sb if f < NPAIR else k_sb
                if ts == NTS - 1:
                    nc.vector.tensor_copy(
                        dst[:, f % NPAIR, ts * 512:(ts + 1) * 512], pq)
                else:
                    nc.scalar.copy(
                        dst[:, f % NPAIR, ts * 512:(ts + 1) * 512], pq)

            # v, token-major: out [tok128, 512]
            for tt4 in range(4):
                tt = ts * 4 + tt4
                pv = ps_v.tile([128, 512], F32)
                for cc in range(NCC):
                    nc.tensor.matmul(
                        pv,
                        x_t[:, cc, tt4 * 128:(tt4 + 1) * 128],
                        w_v[:, cc, :],
                        start=(cc == 0), stop=(cc == NCC - 1),
                    )
                if ts == NTS - 1:
                    nc.vector.tensor_copy(
                        v_sb[:, tt, :, 0:64],
                        pv.rearrange("p (h d) -> p h d", h=HG))
                else:
                    nc.scalar.copy(
                        v_sb[:, tt, :, 0:64],
                        pv.rearrange("p (h d) -> p h d", h=HG))


def self_attention(nc, tc, q_sb, k_sb, v_sb, o_sb, dmask, ps_st, nrm_scr):
    with tc.tile_pool(name="pt_p", bufs=4) as pt_p, \
         tc.tile_pool(name="sum_p", bufs=4) as sum_p, \
         tc.tile_pool(name="rb_p", bufs=6) as rb_p, \
         tc.tile_pool(name="ps_av", bufs=4, space="PSUM") as ps_av:
        pending = []   # deferred normalization multiplies (one pair behind)
        ring = 0
        for qs in range(NTS):
            for pair in range(NPAIR):
                qA = q_sb[0:64, pair, :]
                qB = q_sb[64:128, pair, :]
                kA = k_sb[0:64, pair, :]
                kB = k_sb[64:128, pair, :]
                oA = ps_av.tile([65, 512], F32, tag="av")
                oB = ps_av.tile([65, 512], F32, tag="av")
                # diagonal k-tiles first: their exp+mask latency hides behind
                # the dependency-free full tiles that follow
                kts = list(range(4 * qs, 4 * qs + 4)) + list(range(4 * qs))
                for i, kt in enumerate(kts):
                    d = kt - 4 * qs
                    lo = max(0, d) * 128     # first valid column
                    qsl = slice(qs * 512 + lo, (qs + 1) * 512)
                    ksl = slice(kt * 128, (kt + 1) * 128)
                    st = ps_st.tile([128, 1024], F32, tag="st")
                    nc.tensor.matmul(
                        st[:, lo:512], kA[:, ksl], qA[:, qsl],
                        start=True, stop=True, tile_position=(0, 0))
                    nc.tensor.matmul(
                        st[:, 512 + lo:1024], kB[:, ksl], qB[:, qsl],
                        start=True, stop=True, tile_position=(64, 0))
                    st3 = st.rearrange("p (g n) -> p g n", g=2)
                    pt = pt_p.tile([128, 2, 512], BF16)
                    nc.scalar.activation(
                        pt[:, :, lo:512], st3[:, :, lo:512],
                        mybir.ActivationFunctionType.Exp, scale=float(SCALE))
                    if d >= 0:
                        # zero the true-diagonal block where k>q (r>c)
                        nc.vector.tensor_mul(
                            pt[:, :, d * 128:(d + 1) * 128],
                            pt[:, :, d * 128:(d + 1) * 128],
                            dmask)
                    last = len(kts) - 1
                    nc.tensor.matmul(
                        oA[:, lo:512], v_sb[:, kt, 2 * pair, :],
                        pt[:, 0, lo:512],
                        start=(i == 0), stop=(i == last))
                    nc.tensor.matmul(
                        oB[:, lo:512], v_sb[:, kt, 2 * pair + 1, :],
                        pt[:, 1, lo:512],
                        start=(i == 0), stop=(i == last))
                    if i == 0:
                        # previous pair's norm factors are ready by now
                        for dst, rbs in pending:
                            nc.vector.tensor_mul(dst, dst, rbs)
                        pending = []
                # normalize: approx-reciprocal of the ones-column row sums,
                # broadcast via a DRAM-bounce DMA (keeps DVE chain short)
                osl = slice(qs * 512, (qs + 1) * 512)
                for h, ot in ((0, oA), (1, oB)):
                    nc.vector.tensor_copy(
                        o_sb[64 * h:64 * (h + 1), pair, osl], ot[0:64, :])
                    rt0 = sum_p.tile([1, 512], F32, tag="rt0")
                    nc.vector.tensor_copy(rt0, ot[64:65, :])
                    rt = sum_p.tile([1, 512], F32, tag="rt")
                    nc.vector.reciprocal_approx_fast(rt, rt0)
                    row = nrm_scr[ring % 8:ring % 8 + 1, :]
                    ring += 1
                    nc.sync.dma_start(out=row, in_=rt)
                    rb = rb_p.tile([128, 512], F32)
                    bc = bass.AP(tensor=row.tensor, offset=row.offset,
                                 ap=[[0, 128]] + row.ap[1:])
                    nc.sync.dma_start(out=rb, in_=bc)
                    dst = o_sb[64 * h:64 * (h + 1), pair, osl]
                    pending.append((dst, rb[64 * h:64 * (h + 1), :]))
        for dst, rbs in pending:
            nc.vector.tensor_mul(dst, dst, rbs)


def out_projection(nc, tc, o_sb, w_o, out, ps_st):
    with tc.tile_pool(name="stage_p", bufs=3) as stage_p:
        for tt in range(NTT):
            po = ps_st.tile([128, 1024], F32, tag="st")
            for half in range(2):
                for ccp in range(NPAIR):
                    nc.tensor.matmul(
                        po[:, half * 512:(half + 1) * 512],
                        o_sb[:, ccp, tt * 128:(tt + 1) * 128],
                        w_o[:, ccp, half * 512:(half + 1) * 512],
                        start=(ccp == 0), stop=(ccp == NPAIR - 1),
                    )
            og = stage_p.tile([128, 1024], F32)
            nc.scalar.copy(og, po)
            nc.sync.dma_start(out=out[tt * 128:(tt + 1) * 128, :], in_=og)


def build():
    nc = bacc.Bacc("TRN2", num_devices=8)

    xT = nc.dram_tensor("xT", [C, T], BF16, kind="ExternalInput")          # x[b].T
    wqk = nc.dram_tensor("wqk", [C, 1024], BF16, kind="ExternalInput")     # [c, q512|k512]
    wv = nc.dram_tensor("wv", [C, 512], BF16, kind="ExternalInput")        # [c, v512]
    wo = nc.dram_tensor("wo", [128, NPAIR, 1024], BF16, kind="ExternalInput")
    out = nc.dram_tensor("out", [T, C], F32, kind="ExternalOutput")
    nrm_scr = nc.dram_tensor("nrm_scr", [8, 512], F32).ap()  # bounce rows

    xT_t = xT.ap().rearrange("(cc p) t -> p cc t", p=128)     # [128, 8, T]
    wqk_t = wqk.ap().rearrange("(cc p) f -> p cc f", p=128)   # [128, 8, 1024]
    wv_t = wv.ap().rearrange("(cc p) f -> p cc f", p=128)     # [128, 8, 512]

    with tile.TileContext(nc) as tc:
        with tc.tile_pool(name="persist", bufs=1) as persist:
            q_sb = persist.tile([128, NPAIR, T], BF16)      # [d2, pair, tok]
            k_sb = persist.tile([128, NPAIR, T], BF16)
            v_sb = persist.tile([128, NTT, HG, 65], BF16)   # token-major V_aug
            # multiplicative causal mask for the true-diagonal 128x128 block,
            # replicated twice (head A / head B): 1 where r<=c, else 0
            dmask = persist.tile([128, 2, 128], BF16)
            nc.gpsimd.memset(dmask, 1.0)
            nc.gpsimd.affine_select(
                out=dmask, in_=dmask,
                compare_op=mybir.AluOpType.is_ge,
                fill=0.0, base=0,
                pattern=[[0, 2], [1, 128]],
                channel_multiplier=-1,
            )
            # ones column of V_aug
            nc.gpsimd.memset(v_sb[:, :, :, 64:65], 1.0)

            with tc.tile_pool(name="ps_st", bufs=2, space="PSUM") as ps_st:
                qkv_projection(nc, tc, xT_t, wqk_t, wv_t, q_sb, k_sb, v_sb)

                with tc.tile_pool(name="mid_p", bufs=1) as mid_p:
                    w_o = mid_p.tile([128, NPAIR, 1024], BF16)
                    nc.sync.dma_start(out=w_o, in_=wo[:, :, :])
                    o_sb = mid_p.tile([128, NPAIR, T], BF16)  # normalized out^T
                    self_attention(nc, tc, q_sb, k_sb, v_sb, o_sb, dmask, ps_st, nrm_scr)
                    out_projection(nc, tc, o_sb, w_o, out, ps_st)

    nc.compile()
    return nc


def kernel(x, w_qkv, w_out, b_out):
    x = np.asarray(x, dtype=np.float32)
    w_qkv = np.asarray(w_qkv, dtype=np.float32)
    w_out = np.asarray(w_out, dtype=np.float32)
    b_out = np.asarray(b_out, dtype=np.float32)

    if "nc" not in _CACHE:
        _CACHE["nc"] = build()
    nc = _CACHE["nc"]

    bf = ml_dtypes.bfloat16
    in_maps = []
    for core in range(8):
        b, hg = core // 2, core % 2
        xT = np.ascontiguousarray(x[b].T).astype(bf)  # [C, T]
        wq = w_qkv[hg * 512:(hg + 1) * 512]
        wk = w_qkv[C + hg * 512:C + (hg + 1) * 512]
        wv = w_qkv[2 * C + hg * 512:2 * C + (hg + 1) * 512]
        wqk = np.ascontiguousarray(np.concatenate([wq, wk], axis=0).T).astype(bf)
        wvT = np.ascontiguousarray(wv.T).astype(bf)  # [C, 512]
        woT = np.ascontiguousarray(
            w_out[:, hg * 512:(hg + 1) * 512].T.reshape(NPAIR, 128, 1024)
            .transpose(1, 0, 2)).astype(bf)  # [128, NPAIR, 1024]
        in_maps.append({"xT": xT, "wqk": wqk, "wv": wvT, "wo": woT})

    res = bass_utils.run_bass_kernel_spmd(
        nc, in_maps, core_ids=list(range(8)),
        trace=_CACHE.get("trace", False),
        trace_cores=_CACHE.get("trace_cores"))
    _CACHE["last_results"] = res

    outp = np.empty((B, T, C), dtype=np.float32)
    for b in range(B):
        outp[b] = res.results[2 * b]["out"] + res.results[2 * b + 1]["out"]
    outp += b_out[None, None, :]
    return outp


# revision 28
# speedup vs baseline: 1.1079x; 1.1079x over previous
"""Multi-head causal attention (B=4, T=2048, C=1024, H=16) on 8 TRN2 NeuronCores.

Sharding: core c <-> (batch b = c//2, head-group hg = c%2 of 8 heads).
Each core: QKV projection for its batch tokens / its 8 heads, flash-style
causal attention in transposed-score layout with unsafe softmax (row sums via
a ones-column in V), and a partial out-projection over its 512 head-dims.
Host pre-transposes/casts inputs (bf16) and sums the two partials per batch.

Matmuls in bf16 (full PE rate, FWL weight loads); accumulation and softmax in
fp32 (PSUM); normalization uses batched reciprocal so PSUM banks free fast.
"""

import numpy as np
import ml_dtypes

import concourse.bass as bass
import concourse.mybir as mybir
import concourse.tile as tile
from concourse import bacc, bass_utils

B, T, C = 4, 2048, 1024
N_HEAD = 16
HEAD_DIM = 64
SCALE = 1.0 / np.sqrt(HEAD_DIM)

HG = 8                  # heads per core
NPAIR = HG // 2         # head pairs per core (a pair shares a 128-partition block)
NTT = T // 128          # token tiles (16)
NTS = T // 512          # token supertiles (4)
NCC = C // 128          # contraction chunks (8)

F32 = mybir.dt.float32
BF16 = mybir.dt.bfloat16

_CACHE = {}


def qkv_projection(nc, tc, xT_t, wqk_t, wv_t, q_sb, k_sb, v_sb):
    with tc.tile_pool(name="wqk_p", bufs=1) as wqk_p, \
         tc.tile_pool(name="wv_p", bufs=1) as wv_p, \
         tc.tile_pool(name="x_p", bufs=3) as x_p, \
         tc.tile_pool(name="ps_qk", bufs=2, space="PSUM") as ps_qk, \
         tc.tile_pool(name="ps_v", bufs=2, space="PSUM") as ps_v:
        w_qk = wqk_p.tile([128, NCC, 1024], BF16)
        nc.sync.dma_start(out=w_qk[:, 0, :], in_=wqk_t[:, 0, :])
        x0 = x_p.tile([128, NCC, 512], BF16, tag="x0")
        nc.sync.dma_start(out=x0, in_=xT_t[:, :, 0:512])
        for cc in range(1, NCC):
            nc.sync.dma_start(out=w_qk[:, cc, :], in_=wqk_t[:, cc, :])
        w_v = wv_p.tile([128, NCC, 512], BF16)
        nc.sync.dma_start(out=w_v, in_=wv_t)
        x_ts = [x0]

        for ts in range(NTS):
            if ts > 0:
                x_t = x_p.tile([128, NCC, 512], BF16, tag=f"x{ts % 3}")
                nc.sync.dma_start(out=x_t,
                                  in_=xT_t[:, :, ts * 512:(ts + 1) * 512])
            else:
                x_t = x_ts[0]

            # q and k, feature-major: out [f128, tok512]
            for f in range(2 * NPAIR):  # 0..3 q-pairs, 4..7 k-pairs
                pq = ps_qk.tile([128, 512], F32)
                for cc in range(NCC):
                    nc.tensor.matmul(
                        pq,
                        w_qk[:, cc, f * 128:(f + 1) * 128],
                        x_t[:, cc, :],
                        start=(cc == 0), stop=(cc == NCC - 1),
                    )
                dst = q_sb if f < NPAIR else k_sb
                if ts == NTS - 1:
                    nc.vector.tensor_copy(
                        dst[:, f % NPAIR, ts * 512:(ts + 1) * 512], pq)
                else:
                    nc.scalar.copy(
                        dst[:, f % NPAIR, ts * 512:(ts + 1) * 512], pq)

            # v, token-major: out [tok128, 512]
            for tt4 in range(4):
                tt = ts * 4 + tt4
                pv = ps_v.tile([128, 512], F32)
                for cc in range(NCC):
                    nc.tensor.matmul(
                        pv,
                        x_t[:, cc, tt4 * 128:(tt4 + 1) * 128],
                        w_v[:, cc, :],
                        start=(cc == 0), stop=(cc == NCC - 1),
                    )
                if ts == NTS - 1:
                    nc.vector.tensor_copy(
                        v_sb[:, tt, :, 0:64],
                        pv.rearrange("p (h d) -> p h d", h=HG))
                else:
                    nc.scalar.copy(
                        v_sb[:, tt, :, 0:64],
                        pv.rearrange("p (h d) -> p h d", h=HG))


def self_attention(nc, tc, q_sb, k_sb, v_sb, o_sb, dmask, ps_st, nrm_scr):
    with tc.tile_pool(name="pt_p", bufs=4) as pt_p, \
         tc.tile_pool(name="sum_p", bufs=4) as sum_p, \
         tc.tile_pool(name="rb_p", bufs=6) as rb_p, \
         tc.tile_pool(name="ps_av", bufs=4, space="PSUM") as ps_av:
        pending = []   # deferred normalization multiplies (one pair behind)
        ring = 0
        for qs in range(NTS):
            for pair in range(NPAIR):
                qA = q_sb[0:64, pair, :]
                qB = q_sb[64:128, pair, :]
                kA = k_sb[0:64, pair, :]
                kB = k_sb[64:128, pair, :]
                oA = ps_av.tile([65, 512], F32, tag="av")
                oB = ps_av.tile([65, 512], F32, tag="av")
                kts = list(range(4 * qs + 4))
                for i, kt in enumerate(kts):
                    d = kt - 4 * qs
                    lo = max(0, d) * 128     # first valid column
                    qsl = slice(qs * 512 + lo, (qs + 1) * 512)
                    ksl = slice(kt * 128, (kt + 1) * 128)
                    st = ps_st.tile([128, 1024], F32, tag="st")
                    nc.tensor.matmul(
                        st[:, lo:512], kA[:, ksl], qA[:, qsl],
                        start=True, stop=True, tile_position=(0, 0))
                    nc.tensor.matmul(
                        st[:, 512 + lo:1024], kB[:, ksl], qB[:, qsl],
                        start=True, stop=True, tile_position=(64, 0))
                    st3 = st.rearrange("p (g n) -> p g n", g=2)
                    pt = pt_p.tile([128, 2, 512], BF16)
                    nc.scalar.activation(
                        pt[:, :, lo:512], st3[:, :, lo:512],
                        mybir.ActivationFunctionType.Exp, scale=float(SCALE))
                    if d >= 0:
                        # zero the true-diagonal block where k>q (r>c)
                        nc.vector.tensor_mul(
                            pt[:, :, d * 128:(d + 1) * 128],
                            pt[:, :, d * 128:(d + 1) * 128],
                            dmask)
                    last = len(kts) - 1
                    nc.tensor.matmul(
                        oA[:, lo:512], v_sb[:, kt, 2 * pair, :],
                        pt[:, 0, lo:512],
                        start=(i == 0), stop=(i == last))
                    nc.tensor.matmul(
                        oB[:, lo:512], v_sb[:, kt, 2 * pair + 1, :],
                        pt[:, 1, lo:512],
                        start=(i == 0), stop=(i == last))
                    if i == 0:
                        # previous pair's norm factors are ready by now
                        for dst, rbs in pending:
                            nc.vector.tensor_mul(dst, dst, rbs)
                        pending = []
                # normalize: approx-reciprocal of the ones-column row sums,
                # broadcast via a DRAM-bounce DMA (keeps DVE chain short)
                osl = slice(qs * 512, (qs + 1) * 512)
                for h, ot in ((0, oA), (1, oB)):
                    nc.vector.tensor_copy(
                        o_sb[64 * h:64 * (h + 1), pair, osl], ot[0:64, :])
                    rt0 = sum_p.tile([1, 512], F32, tag="rt0")
                    nc.vector.tensor_copy(rt0, ot[64:65, :])
                    rt = sum_p.tile([1, 512], F32, tag="rt")
                    nc.vector.reciprocal_approx_fast(rt, rt0)
                    row = nrm_scr[ring % 8:ring % 8 + 1, :]
                    ring += 1
                    nc.sync.dma_start(out=row, in_=rt)
                    rb = rb_p.tile([128, 512], F32)
                    bc = bass.AP(tensor=row.tensor, offset=row.offset,
                                 ap=[[0, 128]] + row.ap[1:])
                    nc.sync.dma_start(out=rb, in_=bc)
                    dst = o_sb[64 * h:64 * (h + 1), pair, osl]
                    pending.append((dst, rb[64 * h:64 * (h + 1), :]))
        for dst, rbs in pending:
            nc.vector.tensor_mul(dst, dst, rbs)


def out_projection(nc, tc, o_sb, w_o, out, ps_st):
    with tc.tile_pool(name="stage_p", bufs=3) as stage_p:
        for tt in range(NTT):
            po = ps_st.tile([128, 1024], F32, tag="st")
            for half in range(2):
                for ccp in range(NPAIR):
                    nc.tensor.matmul(
                        po[:, half * 512:(half + 1) * 512],
                        o_sb[:, ccp, tt * 128:(tt + 1) * 128],
                        w_o[:, ccp, half * 512:(half + 1) * 512],
                        start=(ccp == 0), stop=(ccp == NPAIR - 1),
                    )
            og = stage_p.tile([128, 1024], F32)
            nc.scalar.copy(og, po)
            nc.sync.dma_start(out=out[tt * 128:(tt + 1) * 128, :], in_=og)


def build():
    nc = bacc.Bacc("TRN2", num_devices=8)

    xT = nc.dram_tensor("xT", [C, T], BF16, kind="ExternalInput")          # x[b].T
    wqk = nc.dram_tensor("wqk", [C, 1024], BF16, kind="ExternalInput")     # [c, q512|k512]
    wv = nc.dram_tensor("wv", [C, 512], BF16, kind="ExternalInput")        # [c, v512]
    wo = nc.dram_tensor("wo", [128, NPAIR, 1024], BF16, kind="ExternalInput")
    out = nc.dram_tensor("out", [T, C], F32, kind="ExternalOutput")
    nrm_scr = nc.dram_tensor("nrm_scr", [8, 512], F32).ap()  # bounce rows

    xT_t = xT.ap().rearrange("(cc p) t -> p cc t", p=128)     # [128, 8, T]
    wqk_t = wqk.ap().rearrange("(cc p) f -> p cc f", p=128)   # [128, 8, 1024]
    wv_t = wv.ap().rearrange("(cc p) f -> p cc f", p=128)     # [128, 8, 512]

    with tile.TileContext(nc) as tc:
        with tc.tile_pool(name="persist", bufs=1) as persist:
            q_sb = persist.tile([128, NPAIR, T], BF16)      # [d2, pair, tok]
            k_sb = persist.tile([128, NPAIR, T], BF16)
            v_sb = persist.tile([128, NTT, HG, 65], BF16)   # token-major V_aug
            # multiplicative causal mask for the true-diagonal 128x128 block,
            # replicated twice (head A / head B): 1 where r<=c, else 0
            dmask = persist.tile([128, 2, 128], BF16)
            nc.gpsimd.memset(dmask, 1.0)
            nc.gpsimd.affine_select(
                out=dmask, in_=dmask,
                compare_op=mybir.AluOpType.is_ge,
                fill=0.0, base=0,
                pattern=[[0, 2], [1, 128]],
                channel_multiplier=-1,
            )
            # ones column of V_aug
            nc.gpsimd.memset(v_sb[:, :, :, 64:65], 1.0)

            with tc.tile_pool(name="ps_st", bufs=2, space="PSUM") as ps_st:
                qkv_projection(nc, tc, xT_t, wqk_t, wv_t, q_sb, k_sb, v_sb)

                with tc.tile_pool(name="mid_p", bufs=1) as mid_p:
                    w_o = mid_p.tile([128, NPAIR, 1024], BF16)
                    nc.sync.dma_start(out=w_o, in_=wo[:, :, :])
                    o_sb = mid_p.tile([128, NPAIR, T], BF16)  # normalized out^T
                    self_attention(nc, tc, q_sb, k_sb, v_sb, o_sb, dmask, ps_st, nrm_scr)
                    out_projection(nc, tc, o_sb, w_o, out, ps_st)

    nc.compile()
    return nc


def kernel(x, w_qkv, w_out, b_out):
    x = np.asarray(x, dtype=np.float32)
    w_qkv = np.asarray(w_qkv, dtype=np.float32)
    w_out = np.asarray(w_out, dtype=np.float32)
    b_out = np.asarray(b_out, dtype=np.float32)

    if "nc" not in _CACHE:
        _CACHE["nc"] = build()
    nc = _CACHE["nc"]

    bf = ml_dtypes.bfloat16
    in_maps = []
    for core in range(8):
        b, hg = core // 2, core % 2
        xT = np.ascontiguousarray(x[b].T).astype(bf)  # [C, T]
        wq = w_qkv[hg * 512:(hg + 1) * 512]
        wk = w_qkv[C + hg * 512:C + (hg + 1) * 512]
        wv = w_qkv[2 * C + hg * 512:2 * C + (hg + 1) * 512]
        wqk = np.ascontiguousarray(np.concatenate([wq, wk], axis=0).T).astype(bf)
        wvT = np.ascontiguousarray(wv.T).astype(bf)  # [C, 512]
        woT = np.ascontiguousarray(
            w_out[:, hg * 512:(hg + 1) * 512].T.reshape(NPAIR, 128, 1024)
            .transpose(1, 0, 2)).astype(bf)  # [128, NPAIR, 1024]
        in_maps.append({"xT": xT, "wqk": wqk, "wv": wvT, "wo": woT})

    res = bass_utils.run_bass_kernel_spmd(
        nc, in_maps, core_ids=list(range(8)),
        trace=_CACHE.get("trace", False),
        trace_cores=_CACHE.get("trace_cores"))
    _CACHE["last_results"] = res

    outp = np.empty((B, T, C), dtype=np.float32)
    for b in range(B):
        outp[b] = res.results[2 * b]["out"] + res.results[2 * b + 1]["out"]
    outp += b_out[None, None, :]
    return outp


# revision 29
# speedup vs baseline: 1.1090x; 1.0009x over previous
"""Multi-head causal attention (B=4, T=2048, C=1024, H=16) on 8 TRN2 NeuronCores.

Sharding: core c <-> (batch b = c//2, head-group hg = c%2 of 8 heads).
Each core: QKV projection for its batch tokens / its 8 heads, flash-style
causal attention in transposed-score layout with unsafe softmax (row sums via
a ones-column in V), and a partial out-projection over its 512 head-dims.
Host pre-transposes/casts inputs (bf16) and sums the two partials per batch.

Matmuls in bf16 (full PE rate, FWL weight loads); accumulation and softmax in
fp32 (PSUM); normalization uses batched reciprocal so PSUM banks free fast.
"""

import numpy as np
import ml_dtypes

import concourse.bass as bass
import concourse.mybir as mybir
import concourse.tile as tile
from concourse import bacc, bass_utils

B, T, C = 4, 2048, 1024
N_HEAD = 16
HEAD_DIM = 64
SCALE = 1.0 / np.sqrt(HEAD_DIM)

HG = 8                  # heads per core
NPAIR = HG // 2         # head pairs per core (a pair shares a 128-partition block)
NTT = T // 128          # token tiles (16)
NTS = T // 512          # token supertiles (4)
NCC = C // 128          # contraction chunks (8)

F32 = mybir.dt.float32
BF16 = mybir.dt.bfloat16

_CACHE = {}


def qkv_projection(nc, tc, xT_t, wqk_t, wv_t, q_sb, k_sb, v_sb):
    with tc.tile_pool(name="wqk_p", bufs=1) as wqk_p, \
         tc.tile_pool(name="wv_p", bufs=1) as wv_p, \
         tc.tile_pool(name="x_p", bufs=3) as x_p, \
         tc.tile_pool(name="ps_qk", bufs=2, space="PSUM") as ps_qk, \
         tc.tile_pool(name="ps_v", bufs=2, space="PSUM") as ps_v:
        w_qk = wqk_p.tile([128, NCC, 1024], BF16)
        nc.sync.dma_start(out=w_qk[:, 0, :], in_=wqk_t[:, 0, :])
        x0 = x_p.tile([128, NCC, 512], BF16, tag="x0")
        nc.sync.dma_start(out=x0, in_=xT_t[:, :, 0:512])
        for cc in range(1, NCC):
            nc.sync.dma_start(out=w_qk[:, cc, :], in_=wqk_t[:, cc, :])
        w_v = wv_p.tile([128, NCC, 512], BF16)
        nc.sync.dma_start(out=w_v, in_=wv_t)
        x_ts = [x0]

        for ts in range(NTS):
            if ts > 0:
                x_t = x_p.tile([128, NCC, 512], BF16, tag=f"x{ts % 3}")
                nc.sync.dma_start(out=x_t,
                                  in_=xT_t[:, :, ts * 512:(ts + 1) * 512])
            else:
                x_t = x_ts[0]

            # q and k, feature-major: out [f128, tok512]
            for f in range(2 * NPAIR):  # 0..3 q-pairs, 4..7 k-pairs
                pq = ps_qk.tile([128, 512], F32)
                for cc in range(NCC):
                    nc.tensor.matmul(
                        pq,
                        w_qk[:, cc, f * 128:(f + 1) * 128],
                        x_t[:, cc, :],
                        start=(cc == 0), stop=(cc == NCC - 1),
                    )
                dst = q_sb if f < NPAIR else k_sb
                if ts == NTS - 1 and f % 2 == 0:
                    nc.vector.tensor_copy(
                        dst[:, f % NPAIR, ts * 512:(ts + 1) * 512], pq)
                else:
                    nc.scalar.copy(
                        dst[:, f % NPAIR, ts * 512:(ts + 1) * 512], pq)

            # v, token-major: out [tok128, 512]
            for tt4 in range(4):
                tt = ts * 4 + tt4
                pv = ps_v.tile([128, 512], F32)
                for cc in range(NCC):
                    nc.tensor.matmul(
                        pv,
                        x_t[:, cc, tt4 * 128:(tt4 + 1) * 128],
                        w_v[:, cc, :],
                        start=(cc == 0), stop=(cc == NCC - 1),
                    )
                if ts == NTS - 1 and tt4 % 2 == 0:
                    nc.vector.tensor_copy(
                        v_sb[:, tt, :, 0:64],
                        pv.rearrange("p (h d) -> p h d", h=HG))
                else:
                    nc.scalar.copy(
                        v_sb[:, tt, :, 0:64],
                        pv.rearrange("p (h d) -> p h d", h=HG))


def self_attention(nc, tc, q_sb, k_sb, v_sb, o_sb, dmask, ps_st, nrm_scr):
    with tc.tile_pool(name="pt_p", bufs=4) as pt_p, \
         tc.tile_pool(name="sum_p", bufs=4) as sum_p, \
         tc.tile_pool(name="rb_p", bufs=6) as rb_p, \
         tc.tile_pool(name="ps_av", bufs=4, space="PSUM") as ps_av:
        pending = []   # deferred normalization multiplies (one pair behind)
        ring = 0
        for qs in range(NTS):
            for pair in range(NPAIR):
                qA = q_sb[0:64, pair, :]
                qB = q_sb[64:128, pair, :]
                kA = k_sb[0:64, pair, :]
                kB = k_sb[64:128, pair, :]
                oA = ps_av.tile([65, 512], F32, tag="av")
                oB = ps_av.tile([65, 512], F32, tag="av")
                kts = list(range(4 * qs + 4))
                for i, kt in enumerate(kts):
                    d = kt - 4 * qs
                    lo = max(0, d) * 128     # first valid column
                    qsl = slice(qs * 512 + lo, (qs + 1) * 512)
                    ksl = slice(kt * 128, (kt + 1) * 128)
                    st = ps_st.tile([128, 1024], F32, tag="st")
                    nc.tensor.matmul(
                        st[:, lo:512], kA[:, ksl], qA[:, qsl],
                        start=True, stop=True, tile_position=(0, 0))
                    nc.tensor.matmul(
                        st[:, 512 + lo:1024], kB[:, ksl], qB[:, qsl],
                        start=True, stop=True, tile_position=(64, 0))
                    st3 = st.rearrange("p (g n) -> p g n", g=2)
                    pt = pt_p.tile([128, 2, 512], BF16)
                    nc.scalar.activation(
                        pt[:, :, lo:512], st3[:, :, lo:512],
                        mybir.ActivationFunctionType.Exp, scale=float(SCALE))
                    if d >= 0:
                        # zero the true-diagonal block where k>q (r>c)
                        nc.vector.tensor_mul(
                            pt[:, :, d * 128:(d + 1) * 128],
                            pt[:, :, d * 128:(d + 1) * 128],
                            dmask)
                    last = len(kts) - 1
                    nc.tensor.matmul(
                        oA[:, lo:512], v_sb[:, kt, 2 * pair, :],
                        pt[:, 0, lo:512],
                        start=(i == 0), stop=(i == last))
                    nc.tensor.matmul(
                        oB[:, lo:512], v_sb[:, kt, 2 * pair + 1, :],
                        pt[:, 1, lo:512],
                        start=(i == 0), stop=(i == last))
                    if i == 0:
                        # previous pair's norm factors are ready by now
                        for dst, rbs in pending:
                            nc.vector.tensor_mul(dst, dst, rbs)
                        pending = []
                # normalize: approx-reciprocal of the ones-column row sums,
                # broadcast via a DRAM-bounce DMA (keeps DVE chain short)
                osl = slice(qs * 512, (qs + 1) * 512)
                for h, ot in ((0, oA), (1, oB)):
                    nc.vector.tensor_copy(
                        o_sb[64 * h:64 * (h + 1), pair, osl], ot[0:64, :])
                    rt0 = sum_p.tile([1, 512], F32, tag="rt0")
                    nc.vector.tensor_copy(rt0, ot[64:65, :])
                    rt = sum_p.tile([1, 512], F32, tag="rt")
                    nc.vector.reciprocal_approx_fast(rt, rt0)
                    row = nrm_scr[ring % 8:ring % 8 + 1, :]
                    ring += 1
                    nc.sync.dma_start(out=row, in_=rt)
                    rb = rb_p.tile([128, 512], F32)
                    bc = bass.AP(tensor=row.tensor, offset=row.offset,
                                 ap=[[0, 128]] + row.ap[1:])
                    nc.sync.dma_start(out=rb, in_=bc)
                    dst = o_sb[64 * h:64 * (h + 1), pair, osl]
                    pending.append((dst, rb[64 * h:64 * (h + 1), :]))
        for dst, rbs in pending:
            nc.vector.tensor_mul(dst, dst, rbs)


def out_projection(nc, tc, o_sb, w_o, out, ps_st):
    with tc.tile_pool(name="stage_p", bufs=3) as stage_p:
        for tt in range(NTT):
            po = ps_st.tile([128, 1024], F32, tag="st")
            for half in range(2):
                for ccp in range(NPAIR):
                    nc.tensor.matmul(
                        po[:, half * 512:(half + 1) * 512],
                        o_sb[:, ccp, tt * 128:(tt + 1) * 128],
                        w_o[:, ccp, half * 512:(half + 1) * 512],
                        start=(ccp == 0), stop=(ccp == NPAIR - 1),
                    )
            og = stage_p.tile([128, 1024], F32)
            nc.scalar.copy(og, po)
            nc.sync.dma_start(out=out[tt * 128:(tt + 1) * 128, :], in_=og)


def build():
    nc = bacc.Bacc("TRN2", num_devices=8)

    xT = nc.dram_tensor("xT", [C, T], BF16, kind="ExternalInput")          # x[b].T
    wqk = nc.dram_tensor("wqk", [C, 1024], BF16, kind="ExternalInput")     # [c, q512|k512]
    wv = nc.dram_tensor("wv", [C, 512], BF16, kind="ExternalInput")        # [c, v512]
    wo = nc.dram_tensor("wo", [128, NPAIR, 1024], BF16, kind="ExternalInput")
    out = nc.dram_tensor("out", [T, C], F32, kind="ExternalOutput")
    nrm_scr = nc.dram_tensor("nrm_scr", [8, 512], F32).ap()  # bounce rows

    xT_t = xT.ap().rearrange("(cc p) t -> p cc t", p=128)     # [128, 8, T]
    wqk_t = wqk.ap().rearrange("(cc p) f -> p cc f", p=128)   # [128, 8, 1024]
    wv_t = wv.ap().rearrange("(cc p) f -> p cc f", p=128)     # [128, 8, 512]

    with tile.TileContext(nc) as tc:
        with tc.tile_pool(name="persist", bufs=1) as persist:
            q_sb = persist.tile([128, NPAIR, T], BF16)      # [d2, pair, tok]
            k_sb = persist.tile([128, NPAIR, T], BF16)
            v_sb = persist.tile([128, NTT, HG, 65], BF16)   # token-major V_aug
            # multiplicative causal mask for the true-diagonal 128x128 block,
            # replicated twice (head A / head B): 1 where r<=c, else 0
            dmask = persist.tile([128, 2, 128], BF16)
            nc.gpsimd.memset(dmask, 1.0)
            nc.gpsimd.affine_select(
                out=dmask, in_=dmask,
                compare_op=mybir.AluOpType.is_ge,
                fill=0.0, base=0,
                pattern=[[0, 2], [1, 128]],
                channel_multiplier=-1,
            )
            # ones column of V_aug
            nc.gpsimd.memset(v_sb[:, :, :, 64:65], 1.0)

            with tc.tile_pool(name="ps_st", bufs=2, space="PSUM") as ps_st:
                qkv_projection(nc, tc, xT_t, wqk_t, wv_t, q_sb, k_sb, v_sb)

                with tc.tile_pool(name="mid_p", bufs=1) as mid_p:
                    w_o = mid_p.tile([128, NPAIR, 1024], BF16)
                    nc.sync.dma_start(out=w_o, in_=wo[:, :, :])
                    o_sb = mid_p.tile([128, NPAIR, T], BF16)  # normalized out^T
                    self_attention(nc, tc, q_sb, k_sb, v_sb, o_sb, dmask, ps_st, nrm_scr)
                    out_projection(nc, tc, o_sb, w_o, out, ps_st)

    nc.compile()
    return nc


def kernel(x, w_qkv, w_out, b_out):
    x = np.asarray(x, dtype=np.float32)
    w_qkv = np.asarray(w_qkv, dtype=np.float32)
    w_out = np.asarray(w_out, dtype=np.float32)
    b_out = np.asarray(b_out, dtype=np.float32)

    if "nc" not in _CACHE:
        _CACHE["nc"] = build()
    nc = _CACHE["nc"]

    bf = ml_dtypes.bfloat16
    in_maps = []
    for core in range(8):
        b, hg = core // 2, core % 2
        xT = np.ascontiguousarray(x[b].T).astype(bf)  # [C, T]
        wq = w_qkv[hg * 512:(hg + 1) * 512]
        wk = w_qkv[C + hg * 512:C + (hg + 1) * 512]
        wv = w_qkv[2 * C + hg * 512:2 * C + (hg + 1) * 512]
        wqk = np.ascontiguousarray(np.concatenate([wq, wk], axis=0).T).astype(bf)
        wvT = np.ascontiguousarray(wv.T).astype(bf)  # [C, 512]
        woT = np.ascontiguousarray(
            w_out[:, hg * 512:(hg + 1) * 512].T.reshape(NPAIR, 128, 1024)
            .transpose(1, 0, 2)).astype(bf)  # [128, NPAIR, 1024]
        in_maps.append({"xT": xT, "wqk": wqk, "wv": wvT, "wo": woT})

    res = bass_utils.run_bass_kernel_spmd(
        nc, in_maps, core_ids=list(range(8)),
        trace=_CACHE.get("trace", False),
        trace_cores=_CACHE.get("trace_cores"))
    _CACHE["last_results"] = res

    outp = np.empty((B, T, C), dtype=np.float32)
    for b in range(B):
        outp[b] = res.results[2 * b]["out"] + res.results[2 * b + 1]["out"]
    outp += b_out[None, None, :]
    return outp


# revision 30
# speedup vs baseline: 1.1341x; 1.0227x over previous
"""Multi-head causal attention (B=4, T=2048, C=1024, H=16) on 8 TRN2 NeuronCores.

Sharding: core c <-> (batch b = c//2, head-group hg = c%2 of 8 heads).
Each core: QKV projection for its batch tokens / its 8 heads, flash-style
causal attention in transposed-score layout with unsafe softmax (row sums via
a ones-column in V), and a partial out-projection over its 512 head-dims.
Host pre-transposes/casts inputs (bf16) and sums the two partials per batch.

Matmuls in bf16 (full PE rate, FWL weight loads); accumulation and softmax in
fp32 (PSUM); normalization uses batched reciprocal so PSUM banks free fast.
"""

import numpy as np
import ml_dtypes

import concourse.bass as bass
import concourse.mybir as mybir
import concourse.tile as tile
from concourse import bacc, bass_utils

B, T, C = 4, 2048, 1024
N_HEAD = 16
HEAD_DIM = 64
SCALE = 1.0 / np.sqrt(HEAD_DIM)

HG = 8                  # heads per core
NPAIR = HG // 2         # head pairs per core (a pair shares a 128-partition block)
NTT = T // 128          # token tiles (16)
NTS = T // 512          # token supertiles (4)
NCC = C // 128          # contraction chunks (8)

F32 = mybir.dt.float32
BF16 = mybir.dt.bfloat16

_CACHE = {}


def qkv_projection(nc, tc, xT_t, wqk_t, wv_t, q_sb, k_sb, v_sb):
    with tc.tile_pool(name="wqk_p", bufs=1) as wqk_p, \
         tc.tile_pool(name="wv_p", bufs=1) as wv_p, \
         tc.tile_pool(name="x_p", bufs=3) as x_p, \
         tc.tile_pool(name="ps_qk", bufs=2, space="PSUM") as ps_qk, \
         tc.tile_pool(name="ps_v", bufs=2, space="PSUM") as ps_v:
        w_qk = wqk_p.tile([128, NCC, 1024], BF16)
        nc.sync.dma_start(out=w_qk[:, 0, :], in_=wqk_t[:, 0, :])
        x0 = x_p.tile([128, NCC, 512], BF16, tag="x0")
        nc.sync.dma_start(out=x0, in_=xT_t[:, :, 0:512])
        for cc in range(1, NCC):
            nc.sync.dma_start(out=w_qk[:, cc, :], in_=wqk_t[:, cc, :])
        w_v = wv_p.tile([128, NCC, 512], BF16)
        nc.sync.dma_start(out=w_v, in_=wv_t)
        x_ts = [x0]

        for ts in range(NTS):
            if ts > 0:
                x_t = x_p.tile([128, NCC, 512], BF16, tag=f"x{ts % 3}")
                nc.sync.dma_start(out=x_t,
                                  in_=xT_t[:, :, ts * 512:(ts + 1) * 512])
            else:
                x_t = x_ts[0]

            # q and k, feature-major: out [f128, tok512]
            for f in range(2 * NPAIR):  # 0..3 q-pairs, 4..7 k-pairs
                pq = ps_qk.tile([128, 512], F32)
                for cc in range(NCC):
                    nc.tensor.matmul(
                        pq,
                        w_qk[:, cc, f * 128:(f + 1) * 128],
                        x_t[:, cc, :],
                        start=(cc == 0), stop=(cc == NCC - 1),
                    )
                dst = q_sb if f < NPAIR else k_sb
                if f % 2 == 0:
                    nc.vector.tensor_copy(
                        dst[:, f % NPAIR, ts * 512:(ts + 1) * 512], pq)
                else:
                    nc.scalar.copy(
                        dst[:, f % NPAIR, ts * 512:(ts + 1) * 512], pq)

            # v, token-major: out [tok128, 512]
            for tt4 in range(4):
                tt = ts * 4 + tt4
                pv = ps_v.tile([128, 512], F32)
                for cc in range(NCC):
                    nc.tensor.matmul(
                        pv,
                        x_t[:, cc, tt4 * 128:(tt4 + 1) * 128],
                        w_v[:, cc, :],
                        start=(cc == 0), stop=(cc == NCC - 1),
                    )
                if tt4 % 2 == 0:
                    nc.vector.tensor_copy(
                        v_sb[:, tt, :, 0:64],
                        pv.rearrange("p (h d) -> p h d", h=HG))
                else:
                    nc.scalar.copy(
                        v_sb[:, tt, :, 0:64],
                        pv.rearrange("p (h d) -> p h d", h=HG))


def self_attention(nc, tc, q_sb, k_sb, v_sb, o_sb, dmask, ps_st, nrm_scr):
    with tc.tile_pool(name="pt_p", bufs=6) as pt_p, \
         tc.tile_pool(name="sum_p", bufs=6) as sum_p, \
         tc.tile_pool(name="rb_p", bufs=8) as rb_p, \
         tc.tile_pool(name="ps_av", bufs=4, space="PSUM") as ps_av:
        pending = []   # deferred normalization multiplies (one pair behind)
        ring = 0
        for qs in range(NTS):
            for pair in range(NPAIR):
                qA = q_sb[0:64, pair, :]
                qB = q_sb[64:128, pair, :]
                kA = k_sb[0:64, pair, :]
                kB = k_sb[64:128, pair, :]
                oA = ps_av.tile([65, 512], F32, tag="av")
                oB = ps_av.tile([65, 512], F32, tag="av")
                kts = list(range(4 * qs + 4))
                for i, kt in enumerate(kts):
                    d = kt - 4 * qs
                    lo = max(0, d) * 128     # first valid column
                    qsl = slice(qs * 512 + lo, (qs + 1) * 512)
                    ksl = slice(kt * 128, (kt + 1) * 128)
                    st = ps_st.tile([128, 1024], F32, tag="st")
                    nc.tensor.matmul(
                        st[:, lo:512], kA[:, ksl], qA[:, qsl],
                        start=True, stop=True, tile_position=(0, 0))
                    nc.tensor.matmul(
                        st[:, 512 + lo:1024], kB[:, ksl], qB[:, qsl],
                        start=True, stop=True, tile_position=(64, 0))
                    st3 = st.rearrange("p (g n) -> p g n", g=2)
                    pt = pt_p.tile([128, 2, 512], BF16)
                    nc.scalar.activation(
                        pt[:, :, lo:512], st3[:, :, lo:512],
                        mybir.ActivationFunctionType.Exp, scale=float(SCALE))
                    if d >= 0:
                        # zero the true-diagonal block where k>q (r>c)
                        nc.vector.tensor_mul(
                            pt[:, :, d * 128:(d + 1) * 128],
                            pt[:, :, d * 128:(d + 1) * 128],
                            dmask)
                    last = len(kts) - 1
                    nc.tensor.matmul(
                        oA[:, lo:512], v_sb[:, kt, 2 * pair, :],
                        pt[:, 0, lo:512],
                        start=(i == 0), stop=(i == last))
                    nc.tensor.matmul(
                        oB[:, lo:512], v_sb[:, kt, 2 * pair + 1, :],
                        pt[:, 1, lo:512],
                        start=(i == 0), stop=(i == last))
                    if i == 0:
                        # previous pair's norm factors are ready by now
                        for dst, rbs in pending:
                            nc.vector.tensor_mul(dst, dst, rbs)
                        pending = []
                # normalize: approx-reciprocal of the ones-column row sums,
                # broadcast via a DRAM-bounce DMA (keeps DVE chain short)
                osl = slice(qs * 512, (qs + 1) * 512)
                for h, ot in ((0, oA), (1, oB)):
                    nc.vector.tensor_copy(
                        o_sb[64 * h:64 * (h + 1), pair, osl], ot[0:64, :])
                    rt0 = sum_p.tile([1, 512], F32, tag="rt0")
                    nc.vector.tensor_copy(rt0, ot[64:65, :])
                    rt = sum_p.tile([1, 512], F32, tag="rt")
                    nc.vector.reciprocal_approx_fast(rt, rt0)
                    row = nrm_scr[ring % 8:ring % 8 + 1, :]
                    ring += 1
                    nc.gpsimd.dma_start(out=row, in_=rt)
                    rb = rb_p.tile([128, 512], F32)
                    bc = bass.AP(tensor=row.tensor, offset=row.offset,
                                 ap=[[0, 128]] + row.ap[1:])
                    nc.gpsimd.dma_start(out=rb, in_=bc)
                    dst = o_sb[64 * h:64 * (h + 1), pair, osl]
                    pending.append((dst, rb[64 * h:64 * (h + 1), :]))
        for dst, rbs in pending:
            nc.vector.tensor_mul(dst, dst, rbs)


def out_projection(nc, tc, o_sb, w_o, out, ps_st):
    with tc.tile_pool(name="stage_p", bufs=3) as stage_p:
        for tt in range(NTT):
            po = ps_st.tile([128, 1024], F32, tag="st")
            for half in range(2):
                for ccp in range(NPAIR):
                    nc.tensor.matmul(
                        po[:, half * 512:(half + 1) * 512],
                        o_sb[:, ccp, tt * 128:(tt + 1) * 128],
                        w_o[:, ccp, half * 512:(half + 1) * 512],
                        start=(ccp == 0), stop=(ccp == NPAIR - 1),
                    )
            og = stage_p.tile([128, 1024], F32)
            nc.scalar.copy(og, po)
            nc.sync.dma_start(out=out[tt * 128:(tt + 1) * 128, :], in_=og)


def build():
    nc = bacc.Bacc("TRN2", num_devices=8)

    xT = nc.dram_tensor("xT", [C, T], BF16, kind="ExternalInput")          # x[b].T
    wqk = nc.dram_tensor("wqk", [C, 1024], BF16, kind="ExternalInput")     # [c, q512|k512]
    wv = nc.dram_tensor("wv", [C, 512], BF16, kind="ExternalInput")        # [c, v512]
    wo = nc.dram_tensor("wo", [128, NPAIR, 1024], BF16, kind="ExternalInput")
    out = nc.dram_tensor("out", [T, C], F32, kind="ExternalOutput")
    nrm_scr = nc.dram_tensor("nrm_scr", [8, 512], F32).ap()  # bounce rows

    xT_t = xT.ap().rearrange("(cc p) t -> p cc t", p=128)     # [128, 8, T]
    wqk_t = wqk.ap().rearrange("(cc p) f -> p cc f", p=128)   # [128, 8, 1024]
    wv_t = wv.ap().rearrange("(cc p) f -> p cc f", p=128)     # [128, 8, 512]

    with tile.TileContext(nc) as tc:
        with tc.tile_pool(name="persist", bufs=1) as persist:
            q_sb = persist.tile([128, NPAIR, T], BF16)      # [d2, pair, tok]
            k_sb = persist.tile([128, NPAIR, T], BF16)
            v_sb = persist.tile([128, NTT, HG, 65], BF16)   # token-major V_aug
            # multiplicative causal mask for the true-diagonal 128x128 block,
            # replicated twice (head A / head B): 1 where r<=c, else 0
            dmask = persist.tile([128, 2, 128], BF16)
            nc.gpsimd.memset(dmask, 1.0)
            nc.gpsimd.affine_select(
                out=dmask, in_=dmask,
                compare_op=mybir.AluOpType.is_ge,
                fill=0.0, base=0,
                pattern=[[0, 2], [1, 128]],
                channel_multiplier=-1,
            )
            # ones column of V_aug
            nc.gpsimd.memset(v_sb[:, :, :, 64:65], 1.0)

            with tc.tile_pool(name="ps_st", bufs=2, space="PSUM") as ps_st:
                qkv_projection(nc, tc, xT_t, wqk_t, wv_t, q_sb, k_sb, v_sb)

                with tc.tile_pool(name="mid_p", bufs=1) as mid_p:
                    w_o = mid_p.tile([128, NPAIR, 1024], BF16)
                    nc.sync.dma_start(out=w_o, in_=wo[:, :, :])
                    o_sb = mid_p.tile([128, NPAIR, T], BF16)  # normalized out^T
                    self_attention(nc, tc, q_sb, k_sb, v_sb, o_sb, dmask, ps_st, nrm_scr)
                    out_projection(nc, tc, o_sb, w_o, out, ps_st)

    nc.compile()
    return nc


def kernel(x, w_qkv, w_out, b_out):
    x = np.asarray(x, dtype=np.float32)
    w_qkv = np.asarray(w_qkv, dtype=np.float32)
    w_out = np.asarray(w_out, dtype=np.float32)
    b_out = np.asarray(b_out, dtype=np.float32)

    if "nc" not in _CACHE:
        _CACHE["nc"] = build()
    nc = _CACHE["nc"]

    bf = ml_dtypes.bfloat16
    in_maps = []
    for core in range(8):
        b, hg = core // 2, core % 2
        xT = np.ascontiguousarray(x[b].T).astype(bf)  # [C, T]
        wq = w_qkv[hg * 512:(hg + 1) * 512]
        wk = w_qkv[C + hg * 512:C + (hg + 1) * 512]
        wv = w_qkv[2 * C + hg * 512:2 * C + (hg + 1) * 512]
        wqk = np.ascontiguousarray(np.concatenate([wq, wk], axis=0).T).astype(bf)
        wvT = np.ascontiguousarray(wv.T).astype(bf)  # [C, 512]
        woT = np.ascontiguousarray(
            w_out[:, hg * 512:(hg + 1) * 512].T.reshape(NPAIR, 128, 1024)
            .transpose(1, 0, 2)).astype(bf)  # [128, NPAIR, 1024]
        in_maps.append({"xT": xT, "wqk": wqk, "wv": wvT, "wo": woT})

    res = bass_utils.run_bass_kernel_spmd(
        nc, in_maps, core_ids=list(range(8)),
        trace=_CACHE.get("trace", False),
        trace_cores=_CACHE.get("trace_cores"))
    _CACHE["last_results"] = res

    outp = np.empty((B, T, C), dtype=np.float32)
    for b in range(B):
        outp[b] = res.results[2 * b]["out"] + res.results[2 * b + 1]["out"]
    outp += b_out[None, None, :]
    return outp
